# Initial kernel scaffold
#
"""Trainium2 Bass kernel for masked dot-product attention (nn_DotAttention).

Full-size problem: B=32, S=1024, T=512, D=1024, fp32.
  valid  = arange(S) < lengths[:, None]
  ctx    = context * valid                      # zero padded timesteps
  score  = einsum("btd,bsd->bts", target^T, ctx)
  score  = where(score == 0, -inf, score)       # padded positions dot to exactly 0
  attn   = softmax(score, axis=-1)
  result = einsum("bts,bsd->btd", attn, ctx)
  returns (attn.transpose(1,0,2) [T,B,S], result.transpose(1,0,2) [T,B,D])

Sharding: batch-parallel over 8 NeuronCores, 4 batches per core.

Sparsity: attn/result only depend on context rows s < lengths[b]; everything
past that is exactly zero after the softmax mask. Batches are sorted by
length and dealt round-robin so slot j holds similar lengths on every core,
then ONE SPMD program is specialized per-slot to the max valid s-tile count
of that slot (compile-time). Columns beyond each batch's true length inside
the slot cover are handled by the runtime mask; columns beyond the cover are
never computed (attn tail is memset to 0, matching the reference exactly).

Per-core dataflow (per batch slot, cover = NSb*128 <= S):
  - DMA ctx s-tiles < NSb into an f32r tile (bit-identical bytes; the PE's
    f32r path tolerates unrounded operands -- measured identical error to
    pre-rounded). Used directly as mm2's moving operand.
  - PE-transpose (f32r) 128x128 blocks -> ctxT [d, s<cover] f32r (the
    psum->sbuf DVE copy rounds); mm1 moving operand.
  - DMA target t-tiles, PE-transpose -> tgtT [d, tt, dt, t] f32r (ACT copy).
  - mm1 (f32r): score[t, s<cover] accumulated over 8 d-tiles into PSUM fp32.
  - masked softmax along s over [0, cover): additive -1e38 mask from
    iota >= lengths[b] (equivalent to the reference's `score==0 -> -inf`:
    padded cols dot to exactly 0 and no valid score is exactly 0).
  - PE-transpose unnormalized p -> attnT f32r; mm2 (f32r) over s<cover;
    result = psum * (1/rowsum) on ACT (normalization commutes with mm2).
"""

import numpy as np

import concourse.bacc as bacc
import concourse.mybir as mybir
import concourse.tile as tile
from concourse.bass import ds, ts
from concourse.bass_utils import run_bass_kernel_spmd
from concourse.masks import make_identity

P = 128
B, S, T, D = 32, 1024, 512, 1024
NCORES = 8
BL = B // NCORES          # batches per core
NT = T // P               # t tiles
ND = D // P               # d tiles
NS = S // P               # s tiles

F32 = mybir.dt.float32
F32R = mybir.dt.float32r
I32 = mybir.dt.int32

NEG_BIG = -1.0e38


def mm1_chunks(cov):
    """Split [0, cov) into moving-operand chunks that never cross a 512-elem
    PSUM bank boundary. fp32 moving max is 512."""
    out = []
    o = 0
    while o < cov:
        sz = min(512, cov - o)
        out.append((o, sz))
        o += sz
    return out


def build_program(slot_ns):
    """slot_ns: tuple of BL ints, valid s-tile count per batch slot (2..8)."""
    nc = bacc.Bacc("TRN2", target_bir_lowering=False, debug=False,
                   num_devices=NCORES)

    ctx_d = nc.dram_tensor("context_loc", [BL, S, D], F32, kind="ExternalInput")
    tgt_d = nc.dram_tensor("target_loc", [T, BL, D], F32, kind="ExternalInput")
    len_d = nc.dram_tensor("lengths_loc", [BL], I32, kind="ExternalInput")
    attn_d = nc.dram_tensor("attn_out", [T, BL, S], F32, kind="ExternalOutput")
    res_d = nc.dram_tensor("res_out", [T, BL, D], F32, kind="ExternalOutput")

    ctx_ap = ctx_d.ap()
    tgt_ap = tgt_d.ap()
    len_ap = len_d.ap()
    attn_ap = attn_d.ap()
    res_ap = res_d.ap()

    with tile.TileContext(nc) as tc:
        with (
            tc.tile_pool(name="consts", bufs=1) as consts,
            tc.tile_pool(name="ctx_r", bufs=2) as ctxr_pool,
            tc.tile_pool(name="ctxT", bufs=1) as ctxT_pool,
            tc.tile_pool(name="tgtT", bufs=1) as tgtT_pool,
            tc.tile_pool(name="tgtn", bufs=4) as tgtn_pool,
            tc.tile_pool(name="mask", bufs=2) as mask_pool,
            tc.tile_pool(name="smask", bufs=3) as smask_pool,
            tc.tile_pool(name="pexp", bufs=3) as p_pool,
            tc.tile_pool(name="attn", bufs=2) as attn_pool,
            tc.tile_pool(name="res", bufs=2) as res_pool,
            tc.tile_pool(name="attnT", bufs=3) as attnT_pool,
            tc.tile_pool(name="stats", bufs=8) as stat_pool,
            tc.tile_pool(name="ps_mm1", bufs=4, space="PSUM") as ps_mm1,
            tc.tile_pool(name="ps_mm2", bufs=2, space="PSUM") as ps_mm2,
            tc.tile_pool(name="ps_tp", bufs=2, space="PSUM") as ps_tp,
        ):
            ident = consts.tile([P, P], F32, tag="ident")
            make_identity(nc, ident[:])
            identr = consts.tile([P, P], F32R, tag="identr")
            nc.vector.tensor_copy(identr[:], ident[:])

            iota_f = consts.tile([P, S], F32, tag="iota")
            len_i = consts.tile([P, BL], I32, tag="leni")
            len_f = consts.tile([P, BL], F32, tag="lenf")

            for b in range(BL):
                NSb = slot_ns[b]
                COV = NSb * P
                chunks = mm1_chunks(COV)

                # ---- targetT: [128(d), tt, dt, 128(t)] f32r ----
                tgtT = tgtT_pool.tile([P, NT, ND, P], F32R, tag="tgtT")
                for tt in range(NT):
                    tgt_nat = tgtn_pool.tile([P, D], F32R, tag="tgt_nat")
                    nc.sync.dma_start(
                        out=tgt_nat[:],
                        in_=tgt_ap[ts(tt, P), b, :].bitcast(F32R),
                    )
                    for g in range(ND // 4):
                        tp = ps_tp.tile([P, 4, P], F32R, tag="tp")
                        for k in range(4):
                            dt = g * 4 + k
                            nc.tensor.matmul(
                                tp[:, k, :], tgt_nat[:, ts(dt, P)], identr[:],
                                is_transpose=True, start=(k == 0), stop=(k == 3),
                            )
                        nc.scalar.copy(tgtT[:, tt, ds(g * 4, 4), :], tp[:])

                # ---- ctx: DMA straight into f32r (bit-identical), then
                #      PE-transpose the fp32 view into ctxT ----
                ctx_r = ctxr_pool.tile([P, NSb, D], F32R, tag="ctx_r")
                ctxT = ctxT_pool.tile([P, ND, COV], F32R, tag="ctxT")
                ctx_b = ctx_ap[b].rearrange("(si p) d -> p si d", p=P)
                for g in range((NSb + 3) // 4):
                    gn = min(4, NSb - g * 4)
                    for h0 in range(0, gn, 2):
                        hn = min(2, gn - h0)
                        nc.sync.dma_start(
                            out=ctx_r[:, ds(g * 4 + h0, hn), :],
                            in_=ctx_b[:, ds(g * 4 + h0, hn), :].bitcast(F32R),
                        )
                    for dt in range(ND):
                        tp = ps_tp.tile([P, 4, P], F32R, tag="tp")
                        for k in range(gn):
                            nc.tensor.matmul(
                                tp[:, k, :],
                                ctx_r[:, g * 4 + k, ts(dt, P)],
                                identr[:],
                                is_transpose=True,
                                start=(k == 0), stop=(k == gn - 1),
                            )
                        nc.vector.tensor_copy(
                            ctxT[:, dt, ds(g * 512, gn * P)], tp[:, :gn, :])

                if b == 0:
                    # constants: emitted after batch-0 DMAs so their small
                    # SWDGE transfers don't delay the first data transfers
                    nc.gpsimd.iota(iota_f[:], pattern=[[1, S]], base=0,
                                   channel_multiplier=0,
                                   allow_small_or_imprecise_dtypes=True)
                    nc.gpsimd.dma_start(out=len_i[:],
                                        in_=len_ap.partition_broadcast(P))
                    nc.vector.tensor_copy(len_f[:], len_i[:])

                # additive mask row: (iota >= len_b) * NEG_BIG
                maskneg = mask_pool.tile([P, S], F32, tag="maskneg")
                nc.vector.tensor_scalar(
                    out=maskneg[:, :COV], in0=iota_f[:, :COV],
                    scalar1=len_f[:, b:b + 1], scalar2=NEG_BIG,
                    op0=mybir.AluOpType.is_ge, op1=mybir.AluOpType.mult,
                )

                for tt in range(NT):
                    # ---- mm1: score[t, s<COV], one PSUM bank per chunk so
                    # up to 4 independent chains pipeline on the PE ----
                    smask = smask_pool.tile([P, S], F32, tag="smask")
                    for (o, sz) in chunks:
                        ps1 = ps_mm1.tile([P, 512], F32, tag="ps1")
                        for dt in range(ND):
                            nc.tensor.matmul(
                                ps1[:, :sz],
                                tgtT[:, tt, dt, :],
                                ctxT[:, dt, ds(o, sz)],
                                start=(dt == 0), stop=(dt == ND - 1),
                            )
                        # mask-add this chunk as soon as its chain retires
                        nc.vector.tensor_tensor(
                            out=smask[:, ds(o, sz)], in0=ps1[:, :sz],
                            in1=maskneg[:, ds(o, sz)], op=mybir.AluOpType.add,
                        )
                    negmax = stat_pool.tile([P, 1], F32, tag="negmax")
                    nc.vector.reduce_max(negmax[:], smask[:, :COV],
                                         axis=mybir.AxisListType.X, negate=True)
                    # exp per chunk so attnT transposes of early s-blocks
                    # start before the whole row is exponentiated
                    p = p_pool.tile([P, S], F32R, tag="p")
                    rsp = stat_pool.tile([P, 2], F32, tag="rsp")
                    for ci, (o, sz) in enumerate(chunks):
                        nc.scalar.activation(
                            p[:, ds(o, sz)], smask[:, ds(o, sz)],
                            mybir.ActivationFunctionType.Exp,
                            bias=negmax[:], scale=1.0,
                        )
                        nc.vector.reduce_sum(rsp[:, ci:ci + 1], p[:, ds(o, sz)],
                                             axis=mybir.AxisListType.X)
                    rowsum = stat_pool.tile([P, 1], F32, tag="rowsum")
                    if len(chunks) == 1:
                        nc.vector.tensor_copy(rowsum[:], rsp[:, 0:1])
                    else:
                        nc.vector.tensor_tensor(
                            out=rowsum[:], in0=rsp[:, 0:1], in1=rsp[:, 1:2],
                            op=mybir.AluOpType.add)
                    rinv = stat_pool.tile([P, 1], F32, tag="rinv")
                    nc.vector.reciprocal(rinv[:], rowsum[:])

                    attn_t = attn_pool.tile([P, S], F32, tag="attn_t")
                    nc.vector.tensor_scalar_mul(attn_t[:, :COV], p[:, :COV],
                                                rinv[:])
                    if COV < S:
                        nc.gpsimd.memset(attn_t[:, COV:], 0.0)
                    nc.sync.dma_start(out=attn_ap[ts(tt, P), b, :], in_=attn_t[:])

                    # ---- attnT (transpose of unnormalized p) f32r ----
                    attnT = attnT_pool.tile([P, NSb, P], F32R, tag="attnT")
                    for g in range((NSb + 3) // 4):
                        gn = min(4, NSb - g * 4)
                        tp = ps_tp.tile([P, 4, P], F32R, tag="tp")
                        for k in range(gn):
                            st = g * 4 + k
                            nc.tensor.matmul(
                                tp[:, k, :], p[:, ts(st, P)], identr[:],
                                is_transpose=True,
                                start=(k == 0), stop=(k == gn - 1),
                            )
                        nc.scalar.copy(attnT[:, ds(g * 4, gn), :], tp[:, :gn, :])

                    # ---- mm2: result[t, d] = sum_{s<COV} p ctx, then *rinv.
                    # Per-bank chains + dual-engine scale copies so the next
                    # tile's mm2 can reuse each bank as soon as it's drained.
                    res_t = res_pool.tile([P, D], F32, tag="res_t")
                    for h in range(2):
                        ps2 = ps_mm2.tile([P, 512], F32, tag="ps2")
                        for st in range(NSb):
                            nc.tensor.matmul(
                                ps2[:],
                                attnT[:, st, :],
                                ctx_r[:, st, ds(h * 512, 512)],
                                start=(st == 0), stop=(st == NSb - 1),
                            )
                        if h == 0:
                            nc.scalar.activation(
                                res_t[:, ds(h * 512, 512)], ps2[:],
                                mybir.ActivationFunctionType.Copy, scale=rinv[:],
                            )
                        else:
                            nc.vector.tensor_scalar_mul(
                                res_t[:, ds(h * 512, 512)], ps2[:], rinv[:])
                        # ship each half as soon as its scale copy lands
                        nc.sync.dma_start(
                            out=res_ap[ts(tt, P), b, ds(h * 512, 512)],
                            in_=res_t[:, ds(h * 512, 512)])

    nc.compile()
    return nc


_NC_CACHE = {}


def _get_nc(slot_ns):
    key = tuple(slot_ns)
    if key not in _NC_CACHE:
        _NC_CACHE[key] = build_program(key)
    return _NC_CACHE[key]


def plan(lengths):
    """Sort batches by length desc; slot j of core c gets rank j*NCORES+c.
    Returns (order, slot_ns): order[j*NCORES+c] = batch index."""
    order = np.argsort(-np.asarray(lengths), kind="stable")
    slot_ns = []
    for j in range(BL):
        mx = int(np.asarray(lengths)[order[j * NCORES]])
        slot_ns.append(max(2, -(-mx // P)))
    return order, tuple(slot_ns)


def shard_inputs(context, lengths, target, order):
    in_maps = []
    for c in range(NCORES):
        idx = [int(order[j * NCORES + c]) for j in range(BL)]
        in_maps.append({
            "context_loc": np.ascontiguousarray(context[idx]),
            "target_loc": np.ascontiguousarray(target[:, idx, :]),
            "lengths_loc": np.ascontiguousarray(lengths[idx]),
        })
    return in_maps


def run(context, lengths, target, trace=False):
    order, slot_ns = plan(lengths)
    nc = _get_nc(slot_ns)
    in_maps = shard_inputs(context, lengths, target, order)
    out = run_bass_kernel_spmd(nc, in_maps, core_ids=list(range(NCORES)),
                               trace=trace)
    attn = np.empty((T, B, S), np.float32)
    res = np.empty((T, B, D), np.float32)
    for c in range(NCORES):
        for j in range(BL):
            bi = int(order[j * NCORES + c])
            attn[:, bi, :] = out.results[c]["attn_out"][:, j, :]
            res[:, bi, :] = out.results[c]["res_out"][:, j, :]
    return (attn, res), out


def kernel(context, lengths, target):
    context = np.asarray(context, dtype=np.float32)
    lengths = np.asarray(lengths, dtype=np.int32)
    target = np.asarray(target, dtype=np.float32)
    (attn, res), _ = run(context, lengths, target, trace=False)
    return attn, res



# revision 1
# speedup vs baseline: 1.3777x; 1.3777x over previous
"""Trainium2 Bass kernel for masked dot-product attention (nn_DotAttention).

Full-size problem: B=32, S=1024, T=512, D=1024, fp32.
  valid  = arange(S) < lengths[:, None]
  ctx    = context * valid                      # zero padded timesteps
  score  = einsum("btd,bsd->bts", target^T, ctx)
  score  = where(score == 0, -inf, score)       # padded positions dot to exactly 0
  attn   = softmax(score, axis=-1)
  result = einsum("bts,bsd->btd", attn, ctx)
  returns (attn.transpose(1,0,2) [T,B,S], result.transpose(1,0,2) [T,B,D])

Sharding: batch-parallel over 8 NeuronCores, 4 batches per core.

Sparsity: attn/result only depend on context rows s < lengths[b]; everything
past that is exactly zero after the softmax mask. Batches are sorted by
length and dealt round-robin so slot j holds similar lengths on every core,
then ONE SPMD program is specialized per-slot to the max valid s-tile count
of that slot (compile-time). Columns beyond each batch's true length inside
the slot cover are handled by the runtime mask; columns beyond the cover are
never computed (attn tail is memset to 0, matching the reference exactly).

Per-core dataflow (per batch slot, cover = NSb*128 <= S):
  - DMA ctx s-tiles < NSb into an f32r tile (bit-identical bytes; the PE's
    f32r path tolerates unrounded operands -- measured identical error to
    pre-rounded). Used directly as mm2's moving operand.
  - PE-transpose (f32r) 128x128 blocks -> ctxT [d, s<cover] f32r (the
    psum->sbuf DVE copy rounds); mm1 moving operand.
  - DMA target t-tiles, PE-transpose -> tgtT [d, tt, dt, t] f32r (ACT copy).
  - mm1 (f32r): score[t, s<cover] accumulated over 8 d-tiles into PSUM fp32.
  - masked softmax along s over [0, cover): additive -1e38 mask from
    iota >= lengths[b] (equivalent to the reference's `score==0 -> -inf`:
    padded cols dot to exactly 0 and no valid score is exactly 0).
  - PE-transpose unnormalized p -> attnT f32r; mm2 (f32r) over s<cover;
    result = psum * (1/rowsum) on ACT (normalization commutes with mm2).
"""

import numpy as np

import concourse.bacc as bacc
import concourse.mybir as mybir
import concourse.tile as tile
from concourse.bass import ds, ts
from concourse.bass_utils import run_bass_kernel_spmd
from concourse.masks import make_identity

P = 128
B, S, T, D = 32, 1024, 512, 1024
NCORES = 8
BL = B // NCORES          # batches per core
NT = T // P               # t tiles
ND = D // P               # d tiles
NS = S // P               # s tiles

F32 = mybir.dt.float32
F32R = mybir.dt.float32r
I32 = mybir.dt.int32

NEG_BIG = -1.0e38


def mm1_chunks(cov):
    """Split [0, cov) into moving-operand chunks that never cross a 512-elem
    PSUM bank boundary. fp32 moving max is 512."""
    out = []
    o = 0
    while o < cov:
        sz = min(512, cov - o)
        out.append((o, sz))
        o += sz
    return out


def build_program(slot_ns):
    """slot_ns: tuple of BL ints, valid s-tile count per batch slot (2..8)."""
    nc = bacc.Bacc("TRN2", target_bir_lowering=False, debug=False,
                   num_devices=NCORES)

    ctx_d = nc.dram_tensor("context_loc", [BL, S, D], F32, kind="ExternalInput")
    tgt_d = nc.dram_tensor("target_loc", [T, BL, D], F32, kind="ExternalInput")
    len_d = nc.dram_tensor("lengths_loc", [BL], I32, kind="ExternalInput")
    attn_d = nc.dram_tensor("attn_out", [T, BL, S], F32, kind="ExternalOutput")
    res_d = nc.dram_tensor("res_out", [T, BL, D], F32, kind="ExternalOutput")

    ctx_ap = ctx_d.ap()
    tgt_ap = tgt_d.ap()
    len_ap = len_d.ap()
    attn_ap = attn_d.ap()
    res_ap = res_d.ap()

    with tile.TileContext(nc) as tc:
        with (
            tc.tile_pool(name="consts", bufs=1) as consts,
            tc.tile_pool(name="ctx_r", bufs=2) as ctxr_pool,
            tc.tile_pool(name="ctxT", bufs=1) as ctxT_pool,
            tc.tile_pool(name="tgtT", bufs=1) as tgtT_pool,
            tc.tile_pool(name="tgtn", bufs=4) as tgtn_pool,
            tc.tile_pool(name="mask", bufs=2) as mask_pool,
            tc.tile_pool(name="smask", bufs=3) as smask_pool,
            tc.tile_pool(name="pexp", bufs=3) as p_pool,
            tc.tile_pool(name="attn", bufs=2) as attn_pool,
            tc.tile_pool(name="res", bufs=2) as res_pool,
            tc.tile_pool(name="attnT", bufs=3) as attnT_pool,
            tc.tile_pool(name="stats", bufs=8) as stat_pool,
            tc.tile_pool(name="ps_mm1", bufs=4, space="PSUM") as ps_mm1,
            tc.tile_pool(name="ps_mm2", bufs=2, space="PSUM") as ps_mm2,
            tc.tile_pool(name="ps_tp", bufs=2, space="PSUM") as ps_tp,
        ):
            ident = consts.tile([P, P], F32, tag="ident")
            make_identity(nc, ident[:])
            identr = consts.tile([P, P], F32R, tag="identr")
            nc.vector.tensor_copy(identr[:], ident[:])

            iota_f = consts.tile([P, S], F32, tag="iota")
            len_i = consts.tile([P, BL], I32, tag="leni")
            len_f = consts.tile([P, BL], F32, tag="lenf")

            for b in range(BL):
                NSb = slot_ns[b]
                COV = NSb * P
                chunks = mm1_chunks(COV)

                # ---- targetT: [128(d), tt, dt, 128(t)] f32r ----
                tgtT = tgtT_pool.tile([P, NT, ND, P], F32R, tag="tgtT")
                for tt in range(NT):
                    tgt_nat = tgtn_pool.tile([P, D], F32R, tag="tgt_nat")
                    nc.sync.dma_start(
                        out=tgt_nat[:],
                        in_=tgt_ap[ts(tt, P), b, :].bitcast(F32R),
                    )
                    for g in range(ND // 4):
                        tp = ps_tp.tile([P, 4, P], F32R, tag="tp")
                        for k in range(4):
                            dt = g * 4 + k
                            nc.tensor.matmul(
                                tp[:, k, :], tgt_nat[:, ts(dt, P)], identr[:],
                                is_transpose=True, start=(k == 0), stop=(k == 3),
                            )
                        nc.scalar.copy(tgtT[:, tt, ds(g * 4, 4), :], tp[:])

                # ---- ctx: DMA straight into f32r (bit-identical), then
                #      PE-transpose the fp32 view into ctxT ----
                ctx_r = ctxr_pool.tile([P, NSb, D], F32R, tag="ctx_r")
                ctxT = ctxT_pool.tile([P, ND, COV], F32R, tag="ctxT")
                ctx_b = ctx_ap[b].rearrange("(si p) d -> p si d", p=P)
                for g in range((NSb + 3) // 4):
                    gn = min(4, NSb - g * 4)
                    for h0 in range(0, gn, 2):
                        hn = min(2, gn - h0)
                        nc.sync.dma_start(
                            out=ctx_r[:, ds(g * 4 + h0, hn), :],
                            in_=ctx_b[:, ds(g * 4 + h0, hn), :].bitcast(F32R),
                        )
                    for dt in range(ND):
                        tp = ps_tp.tile([P, 4, P], F32R, tag="tp")
                        for k in range(gn):
                            nc.tensor.matmul(
                                tp[:, k, :],
                                ctx_r[:, g * 4 + k, ts(dt, P)],
                                identr[:],
                                is_transpose=True,
                                start=(k == 0), stop=(k == gn - 1),
                            )
                        nc.vector.tensor_copy(
                            ctxT[:, dt, ds(g * 512, gn * P)], tp[:, :gn, :])

                if b == 0:
                    # constants: emitted after batch-0 DMAs so their small
                    # SWDGE transfers don't delay the first data transfers
                    nc.gpsimd.iota(iota_f[:], pattern=[[1, S]], base=0,
                                   channel_multiplier=0,
                                   allow_small_or_imprecise_dtypes=True)
                    nc.gpsimd.dma_start(out=len_i[:],
                                        in_=len_ap.partition_broadcast(P))
                    nc.vector.tensor_copy(len_f[:], len_i[:])

                # additive mask row: (iota >= len_b) * NEG_BIG
                maskneg = mask_pool.tile([P, S], F32, tag="maskneg")
                nc.vector.tensor_scalar(
                    out=maskneg[:, :COV], in0=iota_f[:, :COV],
                    scalar1=len_f[:, b:b + 1], scalar2=NEG_BIG,
                    op0=mybir.AluOpType.is_ge, op1=mybir.AluOpType.mult,
                )

                for tt in range(NT):
                    # ---- mm1: score[t, s<COV], one PSUM bank per chunk so
                    # up to 4 independent chains pipeline on the PE ----
                    smask = smask_pool.tile([P, S], F32, tag="smask")
                    for (o, sz) in chunks:
                        ps1 = ps_mm1.tile([P, 512], F32, tag="ps1")
                        for dt in range(ND):
                            nc.tensor.matmul(
                                ps1[:, :sz],
                                tgtT[:, tt, dt, :],
                                ctxT[:, dt, ds(o, sz)],
                                start=(dt == 0), stop=(dt == ND - 1),
                            )
                        # mask-add this chunk as soon as its chain retires
                        nc.vector.tensor_tensor(
                            out=smask[:, ds(o, sz)], in0=ps1[:, :sz],
                            in1=maskneg[:, ds(o, sz)], op=mybir.AluOpType.add,
                        )
                    negmax = stat_pool.tile([P, 1], F32, tag="negmax")
                    nc.vector.reduce_max(negmax[:], smask[:, :COV],
                                         axis=mybir.AxisListType.X, negate=True)
                    # exp per chunk so attnT transposes of early s-blocks
                    # start before the whole row is exponentiated
                    p = p_pool.tile([P, S], F32R, tag="p")
                    rsp = stat_pool.tile([P, 2], F32, tag="rsp")
                    for ci, (o, sz) in enumerate(chunks):
                        nc.scalar.activation(
                            p[:, ds(o, sz)], smask[:, ds(o, sz)],
                            mybir.ActivationFunctionType.Exp,
                            bias=negmax[:], scale=1.0,
                        )
                        nc.vector.reduce_sum(rsp[:, ci:ci + 1], p[:, ds(o, sz)],
                                             axis=mybir.AxisListType.X)
                    rowsum = stat_pool.tile([P, 1], F32, tag="rowsum")
                    if len(chunks) == 1:
                        nc.vector.tensor_copy(rowsum[:], rsp[:, 0:1])
                    else:
                        nc.vector.tensor_tensor(
                            out=rowsum[:], in0=rsp[:, 0:1], in1=rsp[:, 1:2],
                            op=mybir.AluOpType.add)
                    rinv = stat_pool.tile([P, 1], F32, tag="rinv")
                    nc.vector.reciprocal(rinv[:], rowsum[:])

                    attn_t = attn_pool.tile([P, S], F32, tag="attn_t")
                    nc.vector.tensor_scalar_mul(attn_t[:, :COV], p[:, :COV],
                                                rinv[:])
                    if COV < S:
                        nc.gpsimd.memset(attn_t[:, COV:], 0.0)
                    nc.sync.dma_start(out=attn_ap[ts(tt, P), b, :], in_=attn_t[:])

                    # ---- attnT (transpose of unnormalized p) f32r ----
                    attnT = attnT_pool.tile([P, NSb, P], F32R, tag="attnT")
                    for g in range((NSb + 3) // 4):
                        gn = min(4, NSb - g * 4)
                        tp = ps_tp.tile([P, 4, P], F32R, tag="tp")
                        for k in range(gn):
                            st = g * 4 + k
                            nc.tensor.matmul(
                                tp[:, k, :], p[:, ts(st, P)], identr[:],
                                is_transpose=True,
                                start=(k == 0), stop=(k == gn - 1),
                            )
                        nc.scalar.copy(attnT[:, ds(g * 4, gn), :], tp[:, :gn, :])

                    # ---- mm2: result[t, d] = sum_{s<COV} p ctx, then *rinv.
                    # Per-bank chains + dual-engine scale copies so the next
                    # tile's mm2 can reuse each bank as soon as it's drained.
                    res_t = res_pool.tile([P, D], F32, tag="res_t")
                    for h in range(2):
                        ps2 = ps_mm2.tile([P, 512], F32, tag="ps2")
                        for st in range(NSb):
                            nc.tensor.matmul(
                                ps2[:],
                                attnT[:, st, :],
                                ctx_r[:, st, ds(h * 512, 512)],
                                start=(st == 0), stop=(st == NSb - 1),
                            )
                        if h == 0:
                            nc.scalar.activation(
                                res_t[:, ds(h * 512, 512)], ps2[:],
                                mybir.ActivationFunctionType.Copy, scale=rinv[:],
                            )
                        else:
                            nc.vector.tensor_scalar_mul(
                                res_t[:, ds(h * 512, 512)], ps2[:], rinv[:])
                        # ship each half as soon as its scale copy lands
                        nc.sync.dma_start(
                            out=res_ap[ts(tt, P), b, ds(h * 512, 512)],
                            in_=res_t[:, ds(h * 512, 512)])

    nc.compile()
    return nc


_NC_CACHE = {}


def _get_nc(slot_ns):
    key = tuple(slot_ns)
    if key not in _NC_CACHE:
        _NC_CACHE[key] = build_program(key)
    return _NC_CACHE[key]


def plan(lengths):
    """Sort batches by length desc; slot j of core c gets rank j*NCORES+c.
    Returns (order, slot_ns): order[j*NCORES+c] = batch index."""
    order = np.argsort(-np.asarray(lengths), kind="stable")
    slot_ns = []
    for j in range(BL):
        mx = int(np.asarray(lengths)[order[j * NCORES]])
        slot_ns.append(max(2, -(-mx // P)))
    return order, tuple(slot_ns)


def shard_inputs(context, lengths, target, order):
    in_maps = []
    for c in range(NCORES):
        idx = [int(order[j * NCORES + c]) for j in range(BL)]
        in_maps.append({
            "context_loc": np.ascontiguousarray(context[idx]),
            "target_loc": np.ascontiguousarray(target[:, idx, :]),
            "lengths_loc": np.ascontiguousarray(lengths[idx]),
        })
    return in_maps


def run(context, lengths, target, trace=False):
    order, slot_ns = plan(lengths)
    nc = _get_nc(slot_ns)
    in_maps = shard_inputs(context, lengths, target, order)
    out = run_bass_kernel_spmd(nc, in_maps, core_ids=list(range(NCORES)),
                               trace=trace)
    attn = np.empty((T, B, S), np.float32)
    res = np.empty((T, B, D), np.float32)
    for c in range(NCORES):
        for j in range(BL):
            bi = int(order[j * NCORES + c])
            attn[:, bi, :] = out.results[c]["attn_out"][:, j, :]
            res[:, bi, :] = out.results[c]["res_out"][:, j, :]
    return (attn, res), out


def kernel(context, lengths, target):
    context = np.asarray(context, dtype=np.float32)
    lengths = np.asarray(lengths, dtype=np.int32)
    target = np.asarray(target, dtype=np.float32)
    (attn, res), _ = run(context, lengths, target, trace=False)
    return attn, res

